# revision 8
# baseline (speedup 1.0000x reference)
"""Trainium2 Bass kernel for nn_AdditiveAttention (B=4, S=512, H=128).

Computation per batch b:
  q_proj = query @ Wq.T + attn_b          [S, H]
  k_proj = key @ Wk.T                     [S, H]
  scores[s,t] = sum_h v_w[h] * tanh(q_proj[s,h] + k_proj[t,h])   [S, S]
  aw = softmax(scores, axis=-1)
  out = aw @ value
Returns (out, aw) matching the reference tuple.

Sharding: 8 cores = batch (4) x query-half (2). Each core handles 256
queries of one batch element against all 512 keys.

Device mapping per core:
  - kT/qT layouts put the feature dim h on partitions.
  - k_proj^T, q_proj^T computed by PE (fp32).
  - Per query s: DVE tensor_scalar add (k_proj^T + q_proj^T[:,s] broadcast),
    batched ACT tanh over Q queries at a time, then a PE "matvec" with v_w
    contracting h.  The matvec uses a shifted-weight trick: a [128, 63]
    weight tile with v_w at column 31; slicing [31-j : 63-j] puts v_w in
    column j so query j of a 32-row PSUM block accumulates its score row at
    partition base+j (matmul outputs must be 32-partition aligned).
  - Softmax: DVE max-reduce (negated) -> ACT exp with bias=-max and fused
    free-dim accumulation -> DVE reciprocal + scalar multiply.
  - Output: PE transpose of aw chunks, then accumulated PE matmuls with
    value chunks.
"""

import numpy as np

B, S, H = 4, 512, 128
NCORES = 8
SHALF = S // 2      # queries per core
GQ = 128            # queries per softmax group
NGROUPS = SHALF // GQ
Q = 8               # queries per ACT tanh batch
NSUB = GQ // Q
VCH = S // H        # value chunks of 128

_CACHE = {}


def _build_program(score_dtype="float32r"):
    from contextlib import ExitStack

    import concourse.bacc as bacc
    import concourse.tile as tile
    from concourse import mybir

    dt = mybir.dt
    F32 = dt.float32
    SDT = getattr(dt, score_dtype)
    AF = mybir.ActivationFunctionType
    AX = mybir.AxisListType
    ALU = mybir.AluOpType

    nc = bacc.Bacc(
        "TRN2", target_bir_lowering=False, debug=False, num_devices=NCORES
    )

    def din(name, shape):
        return nc.dram_tensor(name, shape, F32, kind="ExternalInput").ap()

    qT = din("qT", [H, SHALF])        # query^T for this core's 256 queries
    kT = din("kT", [H, S])            # key^T for this core's batch
    vch = din("vch", [H, VCH, H])     # vch[p, c, h] = value[c*128+p, h]
    wqT = din("wqT", [H, H])          # Wq^T  (so lhsT.T @ x = Wq @ x)
    wkT = din("wkT", [H, H])
    bq = din("bq", [H, 1])            # attn_b column
    # zeros with v_w at column 31, in the score matmul dtype
    zvw = nc.dram_tensor("zvw", [H, 63], SDT, kind="ExternalInput").ap()
    ident = din("ident", [H, H])      # identity for PE transpose

    out = nc.dram_tensor("out", [SHALF, H], F32, kind="ExternalOutput").ap()
    aw = nc.dram_tensor("aw", [SHALF, S], F32, kind="ExternalOutput").ap()

    with ExitStack() as ctx:
        tc = ctx.enter_context(tile.TileContext(nc))

        singles = ctx.enter_context(tc.tile_pool(name="singles", bufs=1))

        kT_sb = singles.tile([H, S], F32)
        nc.sync.dma_start(out=kT_sb[:], in_=kT)
        qT_sb = singles.tile([H, SHALF], F32)
        nc.sync.dma_start(out=qT_sb[:], in_=qT)
        v_sb = singles.tile([H, VCH, H], F32)
        nc.sync.dma_start(out=v_sb[:], in_=vch)
        wqT_sb = singles.tile([H, H], F32)
        nc.sync.dma_start(out=wqT_sb[:], in_=wqT)
        wkT_sb = singles.tile([H, H], F32)
        nc.sync.dma_start(out=wkT_sb[:], in_=wkT)
        bq_sb = singles.tile([H, 1], F32)
        nc.sync.dma_start(out=bq_sb[:], in_=bq)
        zvw_sb = singles.tile([H, 63], SDT)
        nc.sync.dma_start(out=zvw_sb[:], in_=zvw)
        id_sb = singles.tile([H, H], F32)
        nc.sync.dma_start(out=id_sb[:], in_=ident)

        # Projections: kp_sb = Wk @ key^T  [h_out, t], qp_sb = Wq @ q^T + b
        kp_sb = singles.tile([H, S], F32)
        qp_sb = singles.tile([H, SHALF], F32)
        with tc.tile_pool(name="setup_ps", bufs=1, space="PSUM") as setup_ps:
            kp_ps = setup_ps.tile([H, S], F32)
            nc.tensor.matmul(out=kp_ps[:], lhsT=wkT_sb[:], rhs=kT_sb[:],
                             start=True, stop=True)
            nc.vector.tensor_copy(kp_sb[:], kp_ps[:])

            qp_ps = setup_ps.tile([H, SHALF], F32)
            nc.tensor.matmul(out=qp_ps[:], lhsT=wqT_sb[:], rhs=qT_sb[:],
                             start=True, stop=True)
            nc.scalar.activation(qp_sb[:], qp_ps[:], AF.Identity,
                                 bias=bq_sb[:], scale=1.0)

        args_pool = ctx.enter_context(tc.tile_pool(name="args", bufs=2))
        tanh_pool = ctx.enter_context(tc.tile_pool(name="tanh", bufs=2))
        scores_pool = ctx.enter_context(
            tc.tile_pool(name="scores", bufs=2, space="PSUM"))
        smax_pool = ctx.enter_context(tc.tile_pool(name="smax", bufs=2))
        stat_pool = ctx.enter_context(tc.tile_pool(name="stats", bufs=4))
        tp_ps_pool = ctx.enter_context(
            tc.tile_pool(name="tp_ps", bufs=2, space="PSUM"))
        awt_pool = ctx.enter_context(tc.tile_pool(name="awt", bufs=2))
        out_ps_pool = ctx.enter_context(
            tc.tile_pool(name="out_ps", bufs=2, space="PSUM"))
        out_sb_pool = ctx.enter_context(tc.tile_pool(name="out_sb", bufs=2))

        # 32-query groups: each group's scores live in a [32, 512] PSUM tile
        # at partition base 0 (matmul outputs must sit at 32-aligned bases,
        # and walrus rejects fp32r matmuls at nonzero column positions).
        G32 = 32
        for g in range(SHALF // G32):
            scores_ps = scores_pool.tile([G32, S], F32)
            for sub in range(G32 // Q):
                arg_t = args_pool.tile([H, Q, S], F32)
                tanh_t = tanh_pool.tile([H, Q, S], SDT)
                for i in range(Q):
                    sq = g * G32 + sub * Q + i
                    nc.vector.tensor_scalar_add(
                        arg_t[:, i, :], kp_sb[:], qp_sb[:, sq:sq + 1])
                nc.scalar.activation(tanh_t[:], arg_t[:], AF.Tanh)
                for i in range(Q):
                    j = sub * Q + i           # 0..31 within group
                    nc.tensor.matmul(
                        out=scores_ps[:],
                        lhsT=zvw_sb[:, 31 - j:63 - j],
                        rhs=tanh_t[:, i, :],
                        start=(j == 0), stop=(j == G32 - 1))

            # softmax over t (free dim) for 32 query rows
            neg_max = stat_pool.tile([G32, 1], F32)
            nc.vector.tensor_reduce(neg_max[:], scores_ps[:], axis=AX.X,
                                    op=ALU.max, negate=True)
            exp_sb = smax_pool.tile([G32, S], F32)
            sums = stat_pool.tile([G32, 1], F32)
            nc.scalar.activation(exp_sb[:], scores_ps[:], AF.Exp,
                                 bias=neg_max[:], scale=1.0,
                                 accum_out=sums[:])
            recip = stat_pool.tile([G32, 1], F32)
            nc.vector.reciprocal(recip[:], sums[:])
            aw_sb = smax_pool.tile([G32, S], F32)
            nc.vector.tensor_scalar_mul(aw_sb[:], exp_sb[:], recip[:])
            nc.sync.dma_start(out=aw[g * G32:(g + 1) * G32, :], in_=aw_sb[:])

            # out rows = aw @ value, via PE-transposed aw chunks
            out_ps = out_ps_pool.tile([G32, H], F32)
            for c in range(VCH):
                awt_ps = tp_ps_pool.tile([H, G32], F32)
                nc.tensor.transpose(awt_ps[:], aw_sb[:, c * H:(c + 1) * H],
                                    id_sb[:G32, :G32])
                awt_sb = awt_pool.tile([H, G32], F32)
                nc.vector.tensor_copy(awt_sb[:], awt_ps[:])
                nc.tensor.matmul(out=out_ps[:], lhsT=awt_sb[:],
                                 rhs=v_sb[:, c, :],
                                 start=(c == 0), stop=(c == VCH - 1))
            out_sb = out_sb_pool.tile([G32, H], F32)
            nc.vector.tensor_copy(out_sb[:], out_ps[:])
            nc.sync.dma_start(out=out[g * G32:(g + 1) * G32, :], in_=out_sb[:])

    nc.compile()
    return nc


def _get_program(score_dtype="float32r"):
    key = ("prog", score_dtype)
    if key not in _CACHE:
        _CACHE[key] = _build_program(score_dtype)
    return _CACHE[key]


def _make_in_maps(query, key, value, attn_W, attn_b, v_w):
    wqT = np.ascontiguousarray(attn_W[:, :H].T, dtype=np.float32)
    wkT = np.ascontiguousarray(attn_W[:, H:].T, dtype=np.float32)
    bq = np.ascontiguousarray(attn_b.reshape(H, 1), dtype=np.float32)
    zvw = np.zeros((H, 63), dtype=np.float32)
    zvw[:, 31] = v_w[0].astype(np.float32)
    ident = np.eye(H, dtype=np.float32)

    in_maps = []
    for c in range(NCORES):
        b, half = divmod(c, 2)
        s0 = half * SHALF
        qT = np.ascontiguousarray(query[b, s0:s0 + SHALF, :].T,
                                  dtype=np.float32)
        kT = np.ascontiguousarray(key[b].T, dtype=np.float32)
        vch = np.ascontiguousarray(
            value[b].reshape(VCH, H, H).transpose(1, 0, 2), dtype=np.float32)
        in_maps.append({
            "qT": qT, "kT": kT, "vch": vch, "wqT": wqT, "wkT": wkT,
            "bq": bq, "zvw": zvw, "ident": ident,
        })
    return in_maps


def _run(query, key, value, attn_W, attn_b, v_w, score_dtype="float32r",
         trace=False):
    import concourse.bass_utils as bass_utils

    nc = _get_program(score_dtype)
    in_maps = _make_in_maps(query, key, value, attn_W, attn_b, v_w)
    res = bass_utils.run_bass_kernel_spmd(
        nc, in_maps, list(range(NCORES)), trace=trace)

    output = np.empty((B, S, H), np.float32)
    attn = np.empty((B, S, S), np.float32)
    for c in range(NCORES):
        b, half = divmod(c, 2)
        s0 = half * SHALF
        output[b, s0:s0 + SHALF] = res.results[c]["out"]
        attn[b, s0:s0 + SHALF] = res.results[c]["aw"]
    return (output, attn), res


def kernel(query, key, value, attn_W, attn_b, v_w):
    query = np.asarray(query, dtype=np.float32)
    key = np.asarray(key, dtype=np.float32)
    value = np.asarray(value, dtype=np.float32)
    attn_W = np.asarray(attn_W, dtype=np.float32)
    attn_b = np.asarray(attn_b, dtype=np.float32)
    v_w = np.asarray(v_w, dtype=np.float32)
    (output, attn), _ = _run(query, key, value, attn_W, attn_b, v_w)
    return output, attn
